# revision 15
# baseline (speedup 1.0000x reference)
"""Trainium2 Bass kernel for a bidirectional LSTM encoder head.

The model: h = tanh(E[tokens] @ W0 + b0); y_fw/y_bw = bidirectional
length-masked LSTM (relu activation, TF gate order i,g,f,o, forget bias
+1.0); output = concat([y_fw[-1], y_bw[-1]], axis=1) @ P.

Key structural fact: the output only uses the LAST batch element, so we
only scan one sequence per direction.  Core 0 computes the forward scan,
core 1 the backward scan (same program, different weights/token order).
The scan runs exactly L = lengths[-1] steps: masked steps beyond L
produce zero outputs and cannot affect steps < L.

Layout: hidden (300) padded to 384 = 3 chunks of 128 partitions; gates
(4*300) padded to 4*384 = 1536 = 12 column-chunks of 128, gate order
[i, f, o, g] so sigmoid covers columns 0..8 and relu(g) columns 9..11.
"""

import sys

sys.path.insert(0, "/opt/trn_rl_repo")

from contextlib import ExitStack

import ml_dtypes
import numpy as np

import concourse.bacc as bacc
import concourse.bass as bass
import concourse.mybir as mybir
import concourse.tile as tile
from concourse.bass_utils import run_bass_kernel_spmd
from concourse.masks import make_identity

F32 = mybir.dt.float32
BF16 = mybir.dt.bfloat16
I32 = mybir.dt.int32

B, T, V, NE, NF, NR, NC = 128, 512, 50000, 300, 300, 300, 64
HPAD = 384  # padded hidden (3 chunks of 128)
GPAD = 1536  # padded gates (12 chunks of 128)
KC = 3  # hidden/embedding chunks
GC = 12  # gate column chunks
SIG = mybir.ActivationFunctionType.Sigmoid
TANH = mybir.ActivationFunctionType.Tanh


def build_program(L: int) -> bass.Bass:
    nc = bacc.Bacc()

    tok_d = nc.dram_tensor("tok4", [B, 4], I32, kind="ExternalInput")
    e_d = nc.dram_tensor("emb_table", [V, NE], F32, kind="ExternalInput")
    w0_d = nc.dram_tensor("w0t", [128, KC, HPAD], F32, kind="ExternalInput")
    b0_d = nc.dram_tensor("b0t", [128, KC], F32, kind="ExternalInput")
    wx_d = nc.dram_tensor("wxt", [128, KC, GPAD], F32, kind="ExternalInput")
    bias_d = nc.dram_tensor("biast", [128, GC], F32, kind="ExternalInput")
    wh_d = nc.dram_tensor("wht", [128, KC, GPAD], BF16, kind="ExternalInput")
    pp_d = nc.dram_tensor("ppt", [128, KC, NC], BF16, kind="ExternalInput")
    out_d = nc.dram_tensor("out", [NC, T], F32, kind="ExternalOutput")

    with ExitStack() as ctx:
        tc = ctx.enter_context(tile.TileContext(nc))
        const = ctx.enter_context(tc.tile_pool(name="const", bufs=1))
        work = ctx.enter_context(tc.tile_pool(name="work", bufs=2))

        # ---- persistent SBUF tensors -------------------------------------
        w0_sb = const.tile([128, KC, HPAD], F32, tag="w0")
        wx_sb = const.tile([128, KC, GPAD], F32, tag="wx")
        wh_sb = const.tile([128, KC, GPAD], BF16, tag="wh")
        pp_sb = const.tile([128, KC, NC], BF16, tag="pp")
        b0_sb = const.tile([128, KC], F32, tag="b0")
        bias_sb = const.tile([128, GC], F32, tag="bias")
        tok_sb = const.tile([128, 4], I32, tag="tok")
        ident = const.tile([128, 128], F32, tag="ident")
        emb_sb = [
            const.tile([128, NE], F32, tag=f"emb{i}", name=f"emb{i}") for i in range(4)
        ]
        embT = const.tile([128, KC, T], F32, tag="embT")
        hsT = const.tile([128, KC, T], F32, tag="hsT")
        xp = const.tile([128, GC, T], F32, tag="xp")
        ysT = const.tile([128, KC, T], BF16, tag="ysT")
        z_sb = const.tile([128, T], F32, tag="z")

        nc.sync.dma_start(out=w0_sb[:], in_=w0_d[:])
        nc.sync.dma_start(out=wx_sb[:], in_=wx_d[:])
        nc.sync.dma_start(out=wh_sb[:], in_=wh_d[:])
        nc.sync.dma_start(out=pp_sb[:], in_=pp_d[:])
        nc.sync.dma_start(out=b0_sb[:], in_=b0_d[:])
        nc.sync.dma_start(out=bias_sb[:], in_=bias_d[:])
        nc.sync.dma_start(out=tok_sb[:], in_=tok_d[:])
        make_identity(nc, ident[:])

        # zero-init: embT (pad lanes must not be NaN), ysT (t>=L and pads)
        nc.vector.memset(embT[:], 0.0)
        nc.vector.memset(ysT[:], 0.0)

        # ---- embedding gather (rows, scan order) -> transpose ------------
        for i in range(4):
            nc.gpsimd.indirect_dma_start(
                out=emb_sb[i][:],
                out_offset=None,
                in_=e_d[:],
                in_offset=bass.IndirectOffsetOnAxis(ap=tok_sb[:, i : i + 1], axis=0),
            )

        tp_pool = ctx.enter_context(tc.tile_pool(name="tp", bufs=2, space="PSUM"))
        for i in range(4):
            for c in range(KC):
                w = min(NE, 128 * (c + 1)) - 128 * c  # 128,128,44
                tp = tp_pool.tile([128, 128], F32, tag="tp")
                nc.tensor.transpose(
                    out=tp[:w, :],
                    in_=emb_sb[i][:, 128 * c : 128 * c + w],
                    identity=ident[:],
                )
                nc.vector.tensor_copy(
                    out=embT[:w, c, 128 * i : 128 * (i + 1)], in_=tp[:w, :]
                )

        # ---- h = tanh(emb @ W0 + b0), transposed layout ------------------
        mm_pool = ctx.enter_context(tc.tile_pool(name="mm", bufs=2, space="PSUM"))
        for m in range(KC):
            ph = mm_pool.tile([128, T], F32, tag="ph")
            for c in range(KC):
                nc.tensor.matmul(
                    ph[:, :L],
                    lhsT=w0_sb[:, c, 128 * m : 128 * (m + 1)],
                    rhs=embT[:, c, :L],
                    start=(c == 0),
                    stop=(c == KC - 1),
                )
            nc.scalar.activation(
                out=hsT[:, m, :L], in_=ph[:, :L], func=TANH, bias=b0_sb[:, m : m + 1]
            )

        # ---- xpart = hs @ Wx + bias (includes forget bias) ---------------
        for j in range(GC):
            px = mm_pool.tile([128, T], F32, tag="ph")
            for c in range(KC):
                nc.tensor.matmul(
                    px[:, :L],
                    lhsT=wx_sb[:, c, 128 * j : 128 * (j + 1)],
                    rhs=hsT[:, c, :L],
                    start=(c == 0),
                    stop=(c == KC - 1),
                )
            nc.vector.tensor_scalar_add(
                out=xp[:, j, :L], in0=px[:, :L], scalar1=bias_sb[:, j : j + 1]
            )

        # ---- the scan ----------------------------------------------------
        pg_pool = ctx.enter_context(tc.tile_pool(name="pg", bufs=3, space="PSUM"))

        def cell(t, gate12, c_prev):
            """gate12: [128, 12] AP of pre-activation gates (order i,f,o,g).
            Returns this step's c tile. Writes h into ysT[:, :, t]."""
            s = work.tile([128, 9], F32, tag="s")
            r = work.tile([128, 3], F32, tag="r")
            nc.scalar.activation(out=s[:], in_=gate12[:, 0:9], func=SIG)
            nc.vector.tensor_scalar_max(out=r[:], in0=gate12[:, 9:12], scalar1=0.0)
            t1 = work.tile([128, 3], F32, tag="t1")
            nc.vector.tensor_mul(out=t1[:], in0=s[:, 0:3], in1=r[:])
            if c_prev is None:
                cn = t1
            else:
                cn = work.tile([128, 3], F32, tag="cn")
                cm = work.tile([128, 3], F32, tag="cm")
                nc.vector.tensor_mul(out=cm[:], in0=s[:, 3:6], in1=c_prev[:])
                nc.vector.tensor_add(out=cn[:], in0=cm[:], in1=t1[:])
            rc = work.tile([128, 3], F32, tag="rc")
            nc.vector.tensor_scalar_max(out=rc[:], in0=cn[:], scalar1=0.0)
            nc.vector.tensor_mul(out=ysT[:, :, t], in0=rc[:], in1=s[:, 6:9])
            return cn

        c_prev = cell(0, xp[:, :, 0], None)
        for t in range(1, L):
            pg = pg_pool.tile([128, GC], F32, tag="pg")
            for j in range(GC):
                for c in range(KC):
                    nc.tensor.matmul(
                        pg[:, j : j + 1],
                        lhsT=wh_sb[:, c, 128 * j : 128 * (j + 1)],
                        rhs=ysT[:, c, t - 1 : t],
                        start=(c == 0),
                        stop=(c == KC - 1),
                    )
            g0 = work.tile([128, GC], F32, tag="g0")
            nc.vector.tensor_add(out=g0[:], in0=pg[:], in1=xp[:, :, t])
            c_prev = cell(t, g0[:], c_prev)

        # ---- z^T = P_half^T @ ys^T  -> [64, T] ---------------------------
        pz = mm_pool.tile([128, T], F32, tag="ph")
        for c in range(KC):
            nc.tensor.matmul(
                pz[:NC, :],
                lhsT=pp_sb[:, c, :],
                rhs=ysT[:, c, :],
                start=(c == 0),
                stop=(c == KC - 1),
            )
        nc.vector.tensor_copy(out=z_sb[:NC, :], in_=pz[:NC, :])
        nc.sync.dma_start(out=out_d[:], in_=z_sb[:NC, :])

    nc.compile()
    return nc


def _prep_gate_weights(W, b):
    """W: [600, 1200] (rows 0:300 x-part, 300:600 h-part), cols in TF order
    i,g,f,o.  Returns Wx_pad [384,1536] f32, Wh_pad [384,1536] f32,
    bias_pad [1536] f32 with our gate order [i, f, o, g] and +1.0 forget."""
    secs = [0, 600, 900, 300]  # i, f, o, g offsets in the original columns
    Wx = np.zeros((HPAD, GPAD), np.float32)
    Wh = np.zeros((HPAD, GPAD), np.float32)
    bias = np.zeros((GPAD,), np.float32)
    for k, s in enumerate(secs):
        Wx[:NF, 384 * k : 384 * k + 300] = W[:NF, s : s + 300]
        Wh[:NR, 384 * k : 384 * k + 300] = W[NF : NF + NR, s : s + 300]
        bias[384 * k : 384 * k + 300] = b[s : s + 300]
    bias[384 : 384 + 300] += 1.0  # TF BasicLSTMCell forget bias
    return Wx, Wh, bias


def _core_inputs(tokens_ord, E, W0, b0, W, bgate, P_half):
    Wx, Wh, bias = _prep_gate_weights(np.asarray(W, np.float32), np.asarray(bgate))
    W0p = np.zeros((HPAD, HPAD), np.float32)
    W0p[:NE, :NF] = np.asarray(W0, np.float32)
    b0p = np.zeros((HPAD,), np.float32)
    b0p[:NF] = np.asarray(b0, np.float32).reshape(-1)
    Pp = np.zeros((HPAD, NC), np.float32)
    Pp[:NR] = np.asarray(P_half, np.float32)
    def chunked(M, width):  # [384, width] -> [128, KC, width]
        return np.ascontiguousarray(M.reshape(KC, 128, width).transpose(1, 0, 2))

    return {
        "tok4": np.ascontiguousarray(
            np.asarray(tokens_ord, np.int32).reshape(4, 128).T
        ),
        "emb_table": np.ascontiguousarray(np.asarray(E, np.float32)),
        "w0t": chunked(W0p, HPAD),
        "b0t": np.ascontiguousarray(b0p.reshape(KC, 128).T),
        "wxt": chunked(Wx, GPAD),
        "biast": np.ascontiguousarray(bias.reshape(GC, 128).T),
        "wht": chunked(Wh, GPAD).astype(ml_dtypes.bfloat16),
        "ppt": chunked(Pp, NC).astype(ml_dtypes.bfloat16),
    }


def _run(tokens, lengths, E, W0, b0, Wf, bf, Wb, bb, P, trace=False):
    tokens = np.asarray(tokens)
    lengths = np.asarray(lengths)
    L = int(lengths[B - 1])
    t_ar = np.arange(T)
    pos_bw = np.where(t_ar < L, L - 1 - t_ar, t_ar)

    tok_last = np.asarray(tokens[B - 1], np.int32)
    in_fw = _core_inputs(tok_last, E, W0, b0, Wf, bf, P[:NR])
    in_bw = _core_inputs(tok_last[pos_bw], E, W0, b0, Wb, bb, P[NR:])

    nc = build_program(L)
    n_cores = 8
    in_maps = [in_fw, in_bw] + [in_fw] * (n_cores - 2)
    res = run_bass_kernel_spmd(nc, in_maps, list(range(n_cores)), trace=trace)

    z_fw = np.asarray(res.results[0]["out"], np.float32).T  # [T, 64]
    z_bw = np.asarray(res.results[1]["out"], np.float32).T
    out = z_fw + z_bw[pos_bw]
    return out.astype(np.float32), res


def kernel(tokens, lengths, E, W0, b0, Wf, bf, Wb, bb, P):
    out, _ = _run(tokens, lengths, E, W0, b0, Wf, bf, Wb, bb, P)
    return out
